# revision 3
# baseline (speedup 1.0000x reference)
"""Causal multi-head attention (B=2, S=2048, D=1024, H=16, hd=64) on 8 trn2 cores.

Sharding: core = (batch b, head-group g): cores 0-3 -> batch 0, groups 0-3;
cores 4-7 -> batch 1. Each core computes 4 heads of one batch element:
QKV projections for its 256 hd-dims, causal attention, and a partial output
projection (attn_heads @ Wo.T restricted to its hd columns). Host sums the 4
partials per batch (bf16 partials) and adds bo.

v2: fully pipelined over 512-token chunks t: proj(t) -> attention(t) ->
outproj(t-1), with token-sliced x DMA so compute starts ~4us in. All PE
matmuls run bf16 (1 cyc/row, same as f32r, but allows narrow diagonal
score matmuls). Causal mask applied by restricting diagonal score matmuls
to live columns plus a [128,128] 0/1 triangle multiply on the probs (DVE)
-- no more identity-matmul mask adds on the PE. Probs are bf16; masked and
dead prob columns are exact zeros so the PV matmul runs full width.
Denominators come from the V_aug ones-column; reciprocal is taken directly
on the PSUM denominator row (1-lane DVE), bounced via DRAM for the
partition broadcast. PSUM is split into three dedicated rings (proj 2,
scores 2, PV 4 banks) so score/PV/proj phases never contend.
"""
import sys

sys.path.insert(0, "/opt/trn_rl_repo")

import numpy as np
import ml_dtypes

import concourse.bass as bass
import concourse.bacc as bacc
import concourse.tile as tile
import concourse.mybir as mybir
from concourse.bass_utils import run_bass_kernel_spmd

B, S, D, H, HD = 2, 2048, 1024, 16, 64
HPC = 4            # heads per core
HDC = HPC * HD     # 256 hd dims per core
KC = D // 128      # 8 contraction chunks
TQ = S // 512      # 4 q-chunks of 512
SCALE = 1.0 / 8.0  # 1/sqrt(64)

f32 = mybir.dt.float32
f32r = mybir.dt.float32r
bf16 = mybir.dt.bfloat16

_CACHE = {}


def _emit(tc, d, ctx):
    nc = tc.nc
    singles = ctx.enter_context(tc.tile_pool(name="singles", bufs=1))
    xt_pool = ctx.enter_context(tc.tile_pool(name="xt", bufs=2))
    qt_pool = ctx.enter_context(tc.tile_pool(name="qt", bufs=2))
    vtt_pool = ctx.enter_context(tc.tile_pool(name="vtt", bufs=2))
    pr_pool = ctx.enter_context(tc.tile_pool(name="pr", bufs=6))
    norm_pool = ctx.enter_context(tc.tile_pool(name="norm", bufs=2))
    stage_pool = ctx.enter_context(tc.tile_pool(name="stage", bufs=2))
    pp = ctx.enter_context(tc.tile_pool(name="pp", bufs=2, space="PSUM"))
    psc = ctx.enter_context(tc.tile_pool(name="psc", bufs=2, space="PSUM"))
    ppv = ctx.enter_context(tc.tile_pool(name="ppv", bufs=4, space="PSUM"))

    # --- preamble: weights first (needed by first proj), rest trickles ---
    w_sb = {}
    for wnm in ("wk", "wq", "wv"):
        w_sb[wnm] = singles.tile([128, KC, HDC], bf16, tag=wnm, name=wnm)
        nc.sync.dma_start(
            out=w_sb[wnm], in_=d[wnm][:].rearrange("p (kc m) -> p kc m", kc=KC)
        )
    bias_sb = singles.tile([128, 6], f32)
    nc.sync.dma_start(out=bias_sb, in_=d["bias"][:])
    ident = singles.tile([128, 128], f32r)
    nc.sync.dma_start(out=ident, in_=d["ident"][:])
    trimask = singles.tile([128, 128], bf16)
    nc.sync.dma_start(out=trimask, in_=d["trimask"][:])
    wo_sb = singles.tile([128, 2, D], bf16)
    nc.sync.dma_start(out=wo_sb, in_=d["wo"][:].rearrange("p (c o) -> p c o", c=2))

    # persistent attention state
    ktz_sb = singles.tile([128, 2, 2, S], bf16, tag="ktz")
    nc.vector.memset(ktz_sb[64:128, :, 0, :], 0.0)
    nc.vector.memset(ktz_sb[0:64, :, 1, :], 0.0)
    v_sb = [
        singles.tile([128, S // 128, 65], bf16, tag=f"v{h}", name=f"v{h}")
        for h in range(HPC)
    ]
    for h in range(HPC):
        nc.vector.memset(v_sb[h][:, :, 64:65], 1.0)
    attnt_sb = singles.tile([128, 2, S], bf16, tag="attnt")

    xd = {
        nm: d[nm][:].rearrange("(c p) s -> p c s", c=KC) for nm in ("xq", "xk", "xv")
    }

    def proj_cells(t, xt, wnm, bcol, dst_fn):
        """K-outer accumulate over 8 chunks for token-chunk t, per mc half."""
        for mc in range(2):
            cell = pp.tile([128, 512], f32, tag="pp", name=f"cell_{wnm}_{t}_{mc}")
            for c in range(KC):
                nc.tensor.matmul(
                    cell,
                    w_sb[wnm][:, c, mc * 128 : (mc + 1) * 128],
                    xt[:, c, :],
                    start=(c == 0),
                    stop=(c == KC - 1),
                )
            dst_fn(mc, cell)

    def outproj(t):
        for tb in range(4):
            i = 4 * t + tb
            ot = stage_pool.tile([128, 2, 512], bf16, tag="ot")
            for o in range(2):
                po = pp.tile([128, 512], f32, tag="pp", name=f"po_{i}_{o}")
                for c in range(2):
                    nc.tensor.matmul(
                        po,
                        attnt_sb[:, c, i * 128 : (i + 1) * 128],
                        wo_sb[:, c, o * 512 : (o + 1) * 512],
                        start=(c == 0),
                        stop=(c == 1),
                    )
                nc.vector.tensor_copy(out=ot[:, o, :], in_=po)
            nc.sync.dma_start(
                out=d["out"][i * 128 : (i + 1) * 128, :],
                in_=ot[:].rearrange("p a b -> p (a b)"),
            )

    for t in range(TQ):
        tsl = slice(t * 512, (t + 1) * 512)
        # ---- stream x for this token chunk ----
        xts = {}
        for nm in ("xk", "xq", "xv"):
            xts[nm] = xt_pool.tile([128, KC, 512], bf16, tag=nm, name=f"{nm}_{t}")
            nc.sync.dma_start(out=xts[nm], in_=xd[nm][:, :, tsl])

        # ---- projections for chunk t ----
        def k_dst(mc, cell):
            nc.vector.tensor_scalar_add(
                out=ktz_sb[0:64, mc, 0, tsl],
                in0=cell[0:64, :],
                scalar1=bias_sb[0:64, 2 + mc : 3 + mc],
            )
            nc.vector.tensor_scalar_add(
                out=ktz_sb[64:128, mc, 1, tsl],
                in0=cell[64:128, :],
                scalar1=bias_sb[64:128, 2 + mc : 3 + mc],
            )

        qt = qt_pool.tile([128, 2, 512], bf16, tag="qt", name=f"qt_{t}")

        def q_dst(mc, cell):
            nc.vector.tensor_scalar_add(
                out=qt[:, mc, :], in0=cell, scalar1=bias_sb[:, mc : mc + 1]
            )

        vtt = vtt_pool.tile([128, 2, 512], f32r, tag="vtt", name=f"vtt_{t}")

        def v_dst(mc, cell):
            nc.vector.tensor_scalar_add(
                out=vtt[:, mc, :],
                in0=cell,
                scalar1=bias_sb[:, 4 + mc : 5 + mc],
            )

        proj_cells(t, xts["xk"], "wk", 2, k_dst)
        proj_cells(t, xts["xq"], "wq", 0, q_dst)
        proj_cells(t, xts["xv"], "wv", 4, v_dst)

        # ---- V: transpose to natural layout, append to per-head V_aug ----
        for mc in range(2):
            for tb in range(4):
                tp = pp.tile([128, 512], f32r, tag="pp", name=f"tp_{t}_{mc}_{tb}")
                with nc.allow_low_precision(reason="f32r transpose; psum is fp32"):
                    nc.tensor.transpose(
                        tp[:, 0:128], vtt[:, mc, tb * 128 : (tb + 1) * 128], ident
                    )
                for h2 in range(2):
                    h = 2 * mc + h2
                    with nc.allow_low_precision(reason="V in bf16 for PV matmul"):
                        nc.vector.tensor_copy(
                            out=v_sb[h][:, 4 * t + tb, 0:64],
                            in_=tp[:, h2 * 64 : (h2 + 1) * 64],
                        )

        # ---- output projection of previous chunk (attnt ready long ago) ----
        if t > 0:
            outproj(t - 1)

        # ---- attention for chunk t ----
        nkb = 4 * t + 4
        for p in range(2):
            pvt = [
                ppv.tile([128, 512], f32, tag="pv", name=f"pv_{t}_{p}_{h2}")
                for h2 in range(2)
            ]
            pending = None  # software pipeline: PV one kb behind scores
            for kb in range(nkb):
                dg = kb - 4 * t
                lo = 128 * dg if dg > 0 else 0
                prs = []
                for h2 in range(2):
                    scg = psc.tile([128, 512], f32, tag="sc", name=f"sc_{t}_{p}_{kb}_{h2}")
                    nc.tensor.matmul(
                        scg[:, lo:512],
                        ktz_sb[:, p, h2, kb * 128 : (kb + 1) * 128],
                        qt[:, p, lo:512],
                        start=True,
                        stop=True,
                    )
                    pr = pr_pool.tile([128, 512], bf16, tag="pr", name=f"pr_{t}_{p}_{kb}_{h2}")
                    if lo > 0:
                        nc.vector.memset(pr[:, 0:lo], 0.0)
                    nc.scalar.activation(
                        out=pr[:, lo:512],
                        in_=scg[:, lo:512],
                        func=mybir.ActivationFunctionType.Exp,
                        scale=SCALE,
                    )
                    if dg >= 0:
                        nc.vector.tensor_tensor(
                            out=pr[:, lo : lo + 128],
                            in0=pr[:, lo : lo + 128],
                            in1=trimask[:],
                            op=mybir.AluOpType.mult,
                        )
                    prs.append(pr)
                if pending is not None:
                    pkb, pprs = pending
                    for h2 in range(2):
                        nc.tensor.matmul(
                            pvt[h2][0:65, :],
                            v_sb[2 * p + h2][:, pkb, :],
                            pprs[h2][:],
                            start=(pkb == 0),
                            stop=False,
                        )
                pending = (kb, prs)
            pkb, pprs = pending
            for h2 in range(2):
                nc.tensor.matmul(
                    pvt[h2][0:65, :],
                    v_sb[2 * p + h2][:, pkb, :],
                    pprs[h2][:],
                    start=(pkb == 0),
                    stop=True,
                )

            # ---- normalize: recip of denom row, DRAM-bounce broadcast ----
            dn2 = norm_pool.tile([65, 2, 512], f32, tag="dn2")
            with nc.allow_low_precision(reason="softmax denominators, fp32"):
                for h2 in range(2):
                    nc.vector.reciprocal(
                        out=dn2[64:65, h2, :], in_=pvt[h2][64:65, :]
                    )
            nc.sync.dma_start(out=d["nscr"][p, t], in_=dn2[64:65, :, :])
            bc = norm_pool.tile([128, 512], f32, tag="bc")
            for h2 in range(2):
                srcd = d["nscr"][p, t, h2, :]
                rep = bass.AP(
                    tensor=srcd.tensor,
                    offset=srcd.offset,
                    ap=[[0, 64]] + [list(e) for e in srcd.ap],
                )
                nc.sync.dma_start(out=bc[h2 * 64 : (h2 + 1) * 64, :], in_=rep)
            tmpb = norm_pool.tile([64, 512], bf16, tag="tmpb")
            with nc.allow_low_precision(reason="attn in bf16"):
                nc.vector.tensor_tensor(
                    out=attnt_sb[0:64, p, tsl],
                    in0=pvt[0][0:64, :],
                    in1=bc[0:64, :],
                    op=mybir.AluOpType.mult,
                )
                nc.vector.tensor_tensor(
                    out=tmpb[:],
                    in0=pvt[1][0:64, :],
                    in1=bc[64:128, :],
                    op=mybir.AluOpType.mult,
                )
            nc.sync.dma_start(out=attnt_sb[64:128, p, tsl], in_=tmpb[:])
            nc.vector.tensor_scalar_add(
                out=attnt_sb[:, p, tsl],
                in0=attnt_sb[:, p, tsl],
                scalar1=bias_sb[:, 4 + p : 5 + p],
            )

    outproj(TQ - 1)


def _build_nc():
    nc = bacc.Bacc()
    d = {}
    for nm in ("xq", "xk", "xv"):
        d[nm] = nc.declare_dram_parameter(nm, [D, S], bf16, isOutput=False)
    for nm in ("wq", "wk", "wv"):
        d[nm] = nc.declare_dram_parameter(nm, [128, KC * HDC], bf16, isOutput=False)
    d["wo"] = nc.declare_dram_parameter("wo", [128, 2 * D], bf16, isOutput=False)
    d["bias"] = nc.declare_dram_parameter("bias", [128, 6], f32, isOutput=False)
    d["trimask"] = nc.declare_dram_parameter("trimask", [128, 128], bf16, isOutput=False)
    d["ident"] = nc.declare_dram_parameter("ident", [128, 128], f32r, isOutput=False)
    d["out"] = nc.declare_dram_parameter("out", [S, D], bf16, isOutput=True)
    d["nscr"] = nc.dram_tensor("nscr", [2, TQ, 2, 512], f32)
    from contextlib import ExitStack

    with tile.TileContext(nc) as tc:
        with ExitStack() as ctx:
            _emit(tc, d, ctx)
    nc.compile()
    return nc


def _get_nc():
    if "nc" not in _CACHE:
        _CACHE["nc"] = _build_nc()
    return _CACHE["nc"]


def _xarr(xt):
    return np.ascontiguousarray(xt).astype(ml_dtypes.bfloat16)


def _warr(wt):  # [D, HDC] -> [128, KC*HDC] chunk-contiguous
    return np.ascontiguousarray(
        wt.reshape(KC, 128, HDC).transpose(1, 0, 2).reshape(128, KC * HDC)
    ).astype(ml_dtypes.bfloat16)


def _woarr(wt):  # [HDC, D] -> [128, 2*D]
    return np.ascontiguousarray(
        wt.reshape(2, 128, D).transpose(1, 0, 2).reshape(128, 2 * D)
    ).astype(ml_dtypes.bfloat16)


def _host_consts():
    p = np.arange(128)[:, None]
    j = np.arange(128)[None, :]
    trimask = (p <= j).astype(ml_dtypes.bfloat16)
    ident = np.eye(128, dtype=np.float32)
    return trimask, ident


def kernel(trace=False, **inputs):
    q = np.asarray(inputs["q"], np.float32)
    k = np.asarray(inputs["k"], np.float32)
    v = np.asarray(inputs["v"], np.float32)
    Wq = np.asarray(inputs["Wq"], np.float32)
    Wk = np.asarray(inputs["Wk"], np.float32)
    Wv = np.asarray(inputs["Wv"], np.float32)
    Wo = np.asarray(inputs["Wo"], np.float32)
    bq = np.asarray(inputs["bq"], np.float32)
    bk = np.asarray(inputs["bk"], np.float32)
    bv = np.asarray(inputs["bv"], np.float32)
    bo = np.asarray(inputs["bo"], np.float32)
    # inputs["mask"] is the causal tril mask, baked into the kernel.

    trimask, ident = _host_consts()
    nc = _get_nc()
    in_maps = []
    for core in range(8):
        b, g = core // 4, core % 4
        sl = slice(g * HDC, (g + 1) * HDC)
        bias = np.zeros((128, 6), np.float32)
        for col, bvec in ((0, bq), (2, bk), (4, bv)):
            seg = bvec[sl].reshape(2, 128)
            bias[:, col] = seg[0]
            bias[:, col + 1] = seg[1]
        in_maps.append(
            {
                "xq": _xarr(q[b].T),
                "xk": _xarr(k[b].T),
                "xv": _xarr(v[b].T),
                "wq": _warr(Wq[sl, :].T),
                "wk": _warr(Wk[sl, :].T),
                "wv": _warr(Wv[sl, :].T),
                "wo": _woarr(Wo[:, sl].T),
                "bias": bias,
                "trimask": trimask,
                "ident": ident,
            }
        )
    res = run_bass_kernel_spmd(nc, in_maps, core_ids=list(range(8)), trace=trace)
    outs = [np.asarray(r["out"], np.float32) for r in res.results]
    final = np.empty((B, S, D), np.float32)
    for b in range(B):
        final[b] = outs[4 * b] + outs[4 * b + 1] + outs[4 * b + 2] + outs[4 * b + 3]
        final[b] += bo
    if trace:
        kernel.last_exec_time_ns = res.exec_time_ns
        kernel.last_results = res
    return final


# revision 6
# speedup vs baseline: 1.0387x; 1.0387x over previous
"""Causal multi-head attention (B=2, S=2048, D=1024, H=16, hd=64) on 8 trn2 cores.

Sharding: core = (batch b, head-group g): cores 0-3 -> batch 0, groups 0-3;
cores 4-7 -> batch 1. Each core computes 4 heads of one batch element:
QKV projections for its 256 hd-dims, causal attention, and a partial output
projection (attn_heads @ Wo.T restricted to its hd columns). Host sums the 4
partials per batch (bf16 partials) and adds bo.

v3: pipelined over 512-token chunks t: proj(t) -> attention(t) ->
outproj(t-1), token-sliced x DMA (first chunks split so compute starts
~4us in). All PE matmuls bf16. Causal mask = restricted diagonal score
matmuls + [128,2,128] 0/1 triangle multiply on probs (DVE); dead prob
columns memset to zero (GpSimd) so PV runs full width. Scores for both
heads of a pair land in one [128,2,512] PSUM tile -> ONE exp per k-block
(h2-paired) to halve Act instruction+semaphore overhead. PSUM: shared
"gen" ring (4x1 bank: proj cells, V transposes, outproj, PV accumulators)
+ "sc" ring (2x2 banks). Denominator: V_aug ones-column -> copy row 64 ->
DRAM bounce -> partition-broadcast read -> wide reciprocal -> fused
multiply into bf16 attnt. QK proj bias-adds run on the (otherwise idle
during proj) Act engine.
"""
import sys

sys.path.insert(0, "/opt/trn_rl_repo")

import numpy as np
import ml_dtypes

import concourse.bass as bass
import concourse.bacc as bacc
import concourse.tile as tile
import concourse.mybir as mybir
from concourse.bass_utils import run_bass_kernel_spmd

B, S, D, H, HD = 2, 2048, 1024, 16, 64
HPC = 4            # heads per core
HDC = HPC * HD     # 256 hd dims per core
KC = D // 128      # 8 contraction chunks
TQ = S // 512      # 4 q-chunks of 512
SCALE = 1.0 / 8.0  # 1/sqrt(64)

f32 = mybir.dt.float32
f32r = mybir.dt.float32r
bf16 = mybir.dt.bfloat16

_CACHE = {}


def _emit(tc, d, ctx):
    nc = tc.nc
    singles = ctx.enter_context(tc.tile_pool(name="singles", bufs=1))
    xt_pool = ctx.enter_context(tc.tile_pool(name="xt", bufs=2))
    qt_pool = ctx.enter_context(tc.tile_pool(name="qt", bufs=2))
    vtt_pool = ctx.enter_context(tc.tile_pool(name="vtt", bufs=2))
    pr_pool = ctx.enter_context(tc.tile_pool(name="pr", bufs=3))
    norm_pool = ctx.enter_context(tc.tile_pool(name="norm", bufs=2))
    stage_pool = ctx.enter_context(tc.tile_pool(name="stage", bufs=2))
    gen = ctx.enter_context(tc.tile_pool(name="gen", bufs=4, space="PSUM"))
    psc = ctx.enter_context(tc.tile_pool(name="psc", bufs=2, space="PSUM"))

    xd = {
        nm: d[nm][:].rearrange("(c p) s -> p c s", c=KC) for nm in ("xq", "xk", "xv")
    }

    # --- preamble DMAs, ordered by first use; x(0) chunk-split for fast start
    w_sb = {}

    def wload(wnm):
        w_sb[wnm] = singles.tile([128, KC, HDC], bf16, tag=wnm, name=wnm)
        nc.sync.dma_start(
            out=w_sb[wnm], in_=d[wnm][:].rearrange("p (kc m) -> p kc m", kc=KC)
        )

    xts0 = {}
    wload("wk")
    xts0["xk"] = xt_pool.tile([128, KC, 512], bf16, tag="xk", name="xk_0")
    for c in range(KC):
        nc.sync.dma_start(out=xts0["xk"][:, c, :], in_=xd["xk"][:, c, 0:512])
    wload("wq")
    xts0["xq"] = xt_pool.tile([128, KC, 512], bf16, tag="xq", name="xq_0")
    for c in range(KC):
        nc.sync.dma_start(out=xts0["xq"][:, c, :], in_=xd["xq"][:, c, 0:512])
    bias_sb = singles.tile([128, 6], f32)
    nc.sync.dma_start(out=bias_sb, in_=d["bias"][:])
    wload("wv")
    xts0["xv"] = xt_pool.tile([128, KC, 512], bf16, tag="xv", name="xv_0")
    nc.sync.dma_start(out=xts0["xv"], in_=xd["xv"][:, :, 0:512])
    trimask = singles.tile([128, 2, 128], bf16)
    nc.sync.dma_start(
        out=trimask, in_=d["trimask"][:].rearrange("p (a b) -> p a b", a=2)
    )
    ident = singles.tile([128, 128], f32r)
    nc.sync.dma_start(out=ident, in_=d["ident"][:])
    wo_sb = singles.tile([128, 2, D], bf16)
    nc.sync.dma_start(out=wo_sb, in_=d["wo"][:].rearrange("p (c o) -> p c o", c=2))

    # persistent attention state
    ktz_sb = singles.tile([128, 2, 2, S], bf16, tag="ktz")
    nc.vector.memset(ktz_sb[64:128, :, 0, :], 0.0)
    nc.vector.memset(ktz_sb[0:64, :, 1, :], 0.0)
    v_sb = [
        singles.tile([128, S // 128, 65], bf16, tag=f"v{h}", name=f"v{h}")
        for h in range(HPC)
    ]
    for h in range(HPC):
        nc.vector.memset(v_sb[h][:, :, 64:65], 1.0)
    attnt_sb = singles.tile([128, 2, S], bf16, tag="attnt")

    def proj_cells(xt, wnm, dst_fn):
        for mc in range(2):
            cell = gen.tile([128, 512], f32, tag="gen", name=f"cell_{wnm}_{mc}")
            for c in range(KC):
                nc.tensor.matmul(
                    cell,
                    w_sb[wnm][:, c, mc * 128 : (mc + 1) * 128],
                    xt[:, c, :],
                    start=(c == 0),
                    stop=(c == KC - 1),
                )
            dst_fn(mc, cell)

    def outproj(t):
        for tb in range(4):
            i = 4 * t + tb
            ot = stage_pool.tile([128, 2, 512], bf16, tag="ot")
            for o in range(2):
                po = gen.tile([128, 512], f32, tag="gen", name=f"po_{i}_{o}")
                for c in range(2):
                    nc.tensor.matmul(
                        po,
                        attnt_sb[:, c, i * 128 : (i + 1) * 128],
                        wo_sb[:, c, o * 512 : (o + 1) * 512],
                        start=(c == 0),
                        stop=(c == 1),
                    )
                nc.vector.tensor_copy(out=ot[:, o, :], in_=po)
            nc.sync.dma_start(
                out=d["out"][i * 128 : (i + 1) * 128, :],
                in_=ot[:].rearrange("p a b -> p (a b)"),
            )

    for t in range(TQ):
        tsl = slice(t * 512, (t + 1) * 512)
        # ---- stream x for this token chunk ----
        if t == 0:
            xts = xts0
        else:
            xts = {}
            for nm in ("xk", "xq", "xv"):
                xts[nm] = xt_pool.tile([128, KC, 512], bf16, tag=nm, name=f"{nm}_{t}")
                nc.sync.dma_start(out=xts[nm], in_=xd[nm][:, :, tsl])

        # ---- projections for chunk t (QK bias-adds on Act engine) ----
        def k_dst(mc, cell):
            nc.scalar.add(
                out=ktz_sb[0:64, mc, 0, tsl],
                in_=cell[0:64, :],
                add=bias_sb[0:64, 2 + mc : 3 + mc],
            )
            nc.scalar.add(
                out=ktz_sb[64:128, mc, 1, tsl],
                in_=cell[64:128, :],
                add=bias_sb[64:128, 2 + mc : 3 + mc],
            )

        qt = qt_pool.tile([128, 2, 512], bf16, tag="qt", name=f"qt_{t}")

        def q_dst(mc, cell):
            nc.scalar.add(
                out=qt[:, mc, :], in_=cell, add=bias_sb[:, mc : mc + 1]
            )

        vtt = vtt_pool.tile([128, 2, 512], f32r, tag="vtt", name=f"vtt_{t}")

        def v_dst(mc, cell):
            nc.vector.tensor_scalar_add(
                out=vtt[:, mc, :],
                in0=cell,
                scalar1=bias_sb[:, 4 + mc : 5 + mc],
            )

        proj_cells(xts["xk"], "wk", k_dst)
        proj_cells(xts["xq"], "wq", q_dst)
        proj_cells(xts["xv"], "wv", v_dst)

        # ---- V: transpose to natural layout, append to per-head V_aug ----
        for mc in range(2):
            for tb in range(4):
                tp = gen.tile([128, 512], f32r, tag="gen", name=f"tp_{t}_{mc}_{tb}")
                with nc.allow_low_precision(reason="f32r transpose; psum is fp32"):
                    nc.tensor.transpose(
                        tp[:, 0:128], vtt[:, mc, tb * 128 : (tb + 1) * 128], ident
                    )
                for h2 in range(2):
                    h = 2 * mc + h2
                    with nc.allow_low_precision(reason="V in bf16 for PV matmul"):
                        nc.vector.tensor_copy(
                            out=v_sb[h][:, 4 * t + tb, 0:64],
                            in_=tp[:, h2 * 64 : (h2 + 1) * 64],
                        )

        # ---- output projection of previous chunk (attnt ready long ago) ----
        if t > 0:
            outproj(t - 1)

        # ---- attention for chunk t ----
        nkb = 4 * t + 4
        for p in range(2):
            pvt = [
                gen.tile([128, 512], f32, tag="gen", name=f"pv_{t}_{p}_{h2}")
                for h2 in range(2)
            ]
            pending = None  # software pipeline: PV one kb behind scores
            for kb in range(nkb):
                dg = kb - 4 * t
                lo = 128 * dg if dg > 0 else 0
                scg = psc.tile([128, 2, 512], f32, tag="sc", name=f"sc_{t}_{p}_{kb}")
                for h2 in range(2):
                    nc.tensor.matmul(
                        scg[:, h2, lo:512],
                        ktz_sb[:, p, h2, kb * 128 : (kb + 1) * 128],
                        qt[:, p, lo:512],
                        start=True,
                        stop=True,
                    )
                pr = pr_pool.tile([128, 2, 512], bf16, tag="pr", name=f"pr_{t}_{p}_{kb}")
                if lo > 0:
                    nc.gpsimd.memset(pr[:, :, 0:lo], 0.0)
                nc.scalar.activation(
                    out=pr[:, :, lo:512],
                    in_=scg[:, :, lo:512],
                    func=mybir.ActivationFunctionType.Exp,
                    scale=SCALE,
                )
                if dg >= 0:
                    nc.vector.tensor_tensor(
                        out=pr[:, :, lo : lo + 128],
                        in0=pr[:, :, lo : lo + 128],
                        in1=trimask[:],
                        op=mybir.AluOpType.mult,
                    )
                if pending is not None:
                    pkb, ppr = pending
                    for h2 in range(2):
                        nc.tensor.matmul(
                            pvt[h2][0:65, :],
                            v_sb[2 * p + h2][:, pkb, :],
                            ppr[:, h2, :],
                            start=(pkb == 0),
                            stop=False,
                        )
                pending = (kb, pr)
            pkb, ppr = pending
            for h2 in range(2):
                nc.tensor.matmul(
                    pvt[h2][0:65, :],
                    v_sb[2 * p + h2][:, pkb, :],
                    ppr[:, h2, :],
                    start=(pkb == 0),
                    stop=True,
                )

            # ---- normalize: denom row -> DRAM bounce -> broadcast -> recip
            dn = norm_pool.tile([65, 2, 512], f32, tag="dn")
            for h2 in range(2):
                nc.vector.tensor_copy(
                    out=dn[64:65, h2, :], in_=pvt[h2][64:65, :]
                )
            nc.sync.dma_start(out=d["nscr"][p, t], in_=dn[64:65, :, :])
            bc = norm_pool.tile([128, 512], f32, tag="bc")
            for h2 in range(2):
                srcd = d["nscr"][p, t, h2, :]
                rep = bass.AP(
                    tensor=srcd.tensor,
                    offset=srcd.offset,
                    ap=[[0, 64]] + [list(e) for e in srcd.ap],
                )
                nc.sync.dma_start(out=bc[h2 * 64 : (h2 + 1) * 64, :], in_=rep)
            with nc.allow_low_precision(reason="softmax denominators, fp32"):
                nc.vector.reciprocal(out=bc[:], in_=bc[:])
            tmpb = norm_pool.tile([64, 512], bf16, tag="tmpb")
            with nc.allow_low_precision(reason="attn in bf16"):
                nc.vector.tensor_tensor(
                    out=attnt_sb[0:64, p, tsl],
                    in0=pvt[0][0:64, :],
                    in1=bc[0:64, :],
                    op=mybir.AluOpType.mult,
                )
                nc.vector.tensor_tensor(
                    out=tmpb[:],
                    in0=pvt[1][0:64, :],
                    in1=bc[64:128, :],
                    op=mybir.AluOpType.mult,
                )
            nc.sync.dma_start(out=attnt_sb[64:128, p, tsl], in_=tmpb[:])
            nc.vector.tensor_scalar_add(
                out=attnt_sb[:, p, tsl],
                in0=attnt_sb[:, p, tsl],
                scalar1=bias_sb[:, 4 + p : 5 + p],
            )

    outproj(TQ - 1)


def _build_nc():
    nc = bacc.Bacc()
    d = {}
    for nm in ("xq", "xk", "xv"):
        d[nm] = nc.declare_dram_parameter(nm, [D, S], bf16, isOutput=False)
    for nm in ("wq", "wk", "wv"):
        d[nm] = nc.declare_dram_parameter(nm, [128, KC * HDC], bf16, isOutput=False)
    d["wo"] = nc.declare_dram_parameter("wo", [128, 2 * D], bf16, isOutput=False)
    d["bias"] = nc.declare_dram_parameter("bias", [128, 6], f32, isOutput=False)
    d["trimask"] = nc.declare_dram_parameter("trimask", [128, 2 * 128], bf16, isOutput=False)
    d["ident"] = nc.declare_dram_parameter("ident", [128, 128], f32r, isOutput=False)
    d["out"] = nc.declare_dram_parameter("out", [S, D], bf16, isOutput=True)
    d["nscr"] = nc.dram_tensor("nscr", [2, TQ, 2, 512], f32)
    from contextlib import ExitStack

    with tile.TileContext(nc) as tc:
        with ExitStack() as ctx:
            _emit(tc, d, ctx)
    nc.compile()
    return nc


def _get_nc():
    if "nc" not in _CACHE:
        _CACHE["nc"] = _build_nc()
    return _CACHE["nc"]


def _xarr(xt):
    return np.ascontiguousarray(xt).astype(ml_dtypes.bfloat16)


def _warr(wt):  # [D, HDC] -> [128, KC*HDC] chunk-contiguous
    return np.ascontiguousarray(
        wt.reshape(KC, 128, HDC).transpose(1, 0, 2).reshape(128, KC * HDC)
    ).astype(ml_dtypes.bfloat16)


def _woarr(wt):  # [HDC, D] -> [128, 2*D]
    return np.ascontiguousarray(
        wt.reshape(2, 128, D).transpose(1, 0, 2).reshape(128, 2 * D)
    ).astype(ml_dtypes.bfloat16)


def _host_consts():
    p = np.arange(128)[:, None]
    j = np.arange(128)[None, :]
    tri = (p <= j).astype(ml_dtypes.bfloat16)
    trimask = np.concatenate([tri, tri], axis=1)  # [128, 2*128], h2-duplicated
    ident = np.eye(128, dtype=np.float32)
    return trimask, ident


def kernel(trace=False, **inputs):
    q = np.asarray(inputs["q"], np.float32)
    k = np.asarray(inputs["k"], np.float32)
    v = np.asarray(inputs["v"], np.float32)
    Wq = np.asarray(inputs["Wq"], np.float32)
    Wk = np.asarray(inputs["Wk"], np.float32)
    Wv = np.asarray(inputs["Wv"], np.float32)
    Wo = np.asarray(inputs["Wo"], np.float32)
    bq = np.asarray(inputs["bq"], np.float32)
    bk = np.asarray(inputs["bk"], np.float32)
    bv = np.asarray(inputs["bv"], np.float32)
    bo = np.asarray(inputs["bo"], np.float32)
    # inputs["mask"] is the causal tril mask, baked into the kernel.

    trimask, ident = _host_consts()
    nc = _get_nc()
    in_maps = []
    for core in range(8):
        b, g = core // 4, core % 4
        sl = slice(g * HDC, (g + 1) * HDC)
        bias = np.zeros((128, 6), np.float32)
        for col, bvec in ((0, bq), (2, bk), (4, bv)):
            seg = bvec[sl].reshape(2, 128)
            bias[:, col] = seg[0]
            bias[:, col + 1] = seg[1]
        in_maps.append(
            {
                "xq": _xarr(q[b].T),
                "xk": _xarr(k[b].T),
                "xv": _xarr(v[b].T),
                "wq": _warr(Wq[sl, :].T),
                "wk": _warr(Wk[sl, :].T),
                "wv": _warr(Wv[sl, :].T),
                "wo": _woarr(Wo[:, sl].T),
                "bias": bias,
                "trimask": trimask,
                "ident": ident,
            }
        )
    res = run_bass_kernel_spmd(nc, in_maps, core_ids=list(range(8)), trace=trace)
    outs = [np.asarray(r["out"], np.float32) for r in res.results]
    final = np.empty((B, S, D), np.float32)
    for b in range(B):
        final[b] = outs[4 * b] + outs[4 * b + 1] + outs[4 * b + 2] + outs[4 * b + 3]
        final[b] += bo
    if trace:
        kernel.last_exec_time_ns = res.exec_time_ns
        kernel.last_results = res
    return final
